# revision 16
# baseline (speedup 1.0000x reference)
"""AFMoE attention layer on 8 NeuronCores (Trainium2, Bass/Tile).

Sharding: core c = (batch b = c//4) x (kv-head group g = c%4).
Each core computes its batch's q-heads 4g..4g+3 + kv head g end-to-end and a
partial output y_c = O_gated @ Wo[:, 512g:512(g+1)].T; the host sums the 4
group partials per batch (row-parallel Wo reduction done on host).

All matmuls run in float32r (full PE rate, ~1e-4 rounding); everything else
is fp32.
"""
import os

import numpy as np

import concourse.bass as bass
import concourse.mybir as mybir
import concourse.tile as tile
from concourse.bass_utils import run_bass_kernel_spmd
from concourse.masks import make_identity

F32 = mybir.dt.float32
F32R = mybir.dt.float32r
AF = mybir.ActivationFunctionType
ALU = mybir.AluOpType
AX = mybir.AxisListType

B, S, H = 2, 2048, 2048
NH, NKV, D = 16, 4, 128
GROUPS = NH // NKV          # q heads per kv head = 4
QH = GROUPS                 # per-core q heads
DQ = QH * D                 # 512
EPS = 1e-5
NT = S // 128               # 16 s-tiles
HC = H // 128               # 16 h-chunks
LAM = float(D) ** -0.5
MAXGRP = 4                  # k-blocks per PSUM score group (x256q = 2 banks)

_nsplit = [0]


def _split_excess_waits(nc, limit=1):
    """This walrus build accepts only one semaphore wait per instruction
    (fp32/fp32r matmuls included). Move excess waits onto preceding
    same-engine NoOps; engine program order keeps this correct."""
    import bass_rust
    for blk in nc.m.functions[0].blocks:
        lst = blk.instructions
        idx = 0
        while idx < len(lst):
            inst = lst[idx]
            si = inst.sync_info
            if (si is None or len(si.on_wait) <= limit
                    or type(inst).__name__ == "InstCollectiveCompute"
                    or inst.engine == mybir.EngineType.Unassigned):
                idx += 1
                continue
            waits = list(si.on_wait)
            kept, excess = waits[-limit:], waits[:-limit]
            new_insts = []
            for w in excess:
                _nsplit[0] += 1
                nop = mybir.InstNoOp(name=f"WS-{_nsplit[0]}", ins=[], outs=[])
                nop.engine = inst.engine
                nop.sync_info = bass_rust.SyncInfo(on_wait=[w], on_update=[])
                new_insts.append(nop)
            inst.sync_info = bass_rust.SyncInfo(on_wait=kept,
                                                on_update=list(si.on_update))
            lst[idx:idx] = new_insts
            idx += len(new_insts) + 1


def _mask_plan(mask2d):
    """Classify the additive mask in [256(q) x 128(k)] slabs (q-tile pairs).

    Returns (rows, mixed_slabs): rows[pair] = list of (kj, mixed_idx|None)
    over a contiguous kj range; mixed_slabs = transposed [128,256] np arrays.
    """
    nb = S // 128
    npair = nb // 2
    uniq = {}
    mixed = []
    rows = []
    for p in range(npair):
        qsl = slice(p * 256, (p + 1) * 256)
        entries = []
        for kj in range(nb):
            blk = mask2d[qsl, kj * 128:(kj + 1) * 128]      # [256 q, 128 k]
            if (blk <= -1e8).all():
                entries.append(None)
            elif (blk == 0.0).all():
                entries.append((kj, None))
            else:
                key = blk.tobytes()
                if key not in uniq:
                    uniq[key] = len(mixed)
                    mixed.append(np.ascontiguousarray(blk.T))
                entries.append((kj, uniq[key]))
        live = [e for e in entries if e is not None]
        if not live:
            raise ValueError("fully-masked query row block unsupported")
        lo = min(e[0] for e in live)
        hi = max(e[0] for e in live)
        row = []
        for kj in range(lo, hi + 1):
            e = entries[kj]
            if e is None:
                blk = mask2d[qsl, kj * 128:(kj + 1) * 128]
                key = blk.tobytes()
                if key not in uniq:
                    uniq[key] = len(mixed)
                    mixed.append(np.ascontiguousarray(blk.T))
                row.append((kj, uniq[key]))
            else:
                row.append(e)
        rows.append(row)
    return rows, mixed


def _build(rows, nmix):
    nc = bass.Bass()
    xt = nc.declare_dram_parameter("xt", [H, S], F32R, isOutput=False)
    wqkv = nc.declare_dram_parameter("wqkv", [H, DQ + 2 * D], F32R, isOutput=False)
    wg = nc.declare_dram_parameter("wg", [H, DQ], F32R, isOutput=False)
    wo = nc.declare_dram_parameter("wo", [DQ, H], F32R, isOutput=False)
    cwq = nc.declare_dram_parameter("cwq", [S, D], F32, isOutput=False)
    swq = nc.declare_dram_parameter("swq", [S, D], F32, isOutput=False)
    cwk = nc.declare_dram_parameter("cwk", [S, D], F32, isOutput=False)
    swk = nc.declare_dram_parameter("swk", [S, D], F32, isOutput=False)
    if nmix:
        maskt = nc.declare_dram_parameter("maskt", [nmix * 128, 256], F32R,
                                          isOutput=False)
    y = nc.declare_dram_parameter("y", [S, H], F32, isOutput=True)

    NW = DQ + 2 * D  # 768

    with tile.TileContext(nc) as tc, \
            nc.allow_low_precision(reason="fp32r matmul operands"), \
            tc.tile_pool(name="const", bufs=1) as const, \
            tc.tile_pool(name="persist", bufs=1) as pp:
        identity_f = const.tile([128, 128], F32)
        make_identity(nc, identity_f)
        if nmix:
            identity_r = const.tile([128, 128], F32R)
            nc.vector.tensor_copy(identity_r, identity_f)
        ones_col_f = const.tile([128, 1], F32)
        nc.vector.memset(ones_col_f, 1.0)
        ones_col = const.tile([128, 1], F32R)
        nc.vector.tensor_copy(ones_col, ones_col_f)
        ones_row_f = const.tile([1, 128], F32)
        nc.vector.memset(ones_row_f, 1.0)
        ones_row = const.tile([1, 128], F32R)
        nc.vector.tensor_copy(ones_row, ones_row_f)
        eps_t = const.tile([128, 1], F32)
        nc.vector.memset(eps_t, EPS)

        qT_all = pp.tile([128, QH, S], F32R)     # [d, h, s]
        kT_all = pp.tile([128, S], F32R)         # [d, s]
        v_all = pp.tile([128, NT, D], F32R)      # [s-part, s-tile, d]
        sigT_all = pp.tile([128, QH, S], F32)    # [d, m, s]

        # ---------------- phase P-a: q/k/v projections + norm + rope ------
        with tc.tile_pool(name="pw", bufs=1) as pw:
          wg_sb = pw.tile([128, HC, DQ], F32R)
          for h in range(HC):
              nc.sync.dma_start(out=wg_sb[:, h, :],
                                in_=wg[h * 128:(h + 1) * 128, :])
          with tc.tile_pool(name="pwq", bufs=1) as pwq, \
                tc.tile_pool(name="pa", bufs=2) as pa, \
                tc.tile_pool(name="psa", bufs=2, space="PSUM") as psa:
            wqkv_sb = pwq.tile([128, HC, NW], F32R)
            for h in range(HC):
                nc.sync.dma_start(out=wqkv_sb[:, h, :],
                                  in_=wqkv[h * 128:(h + 1) * 128, :])
            xt4 = xt.rearrange("(c p) (t q) -> p c t q", p=128, q=128)
            ropes = {}

            def emit_transpose(st):
                qrope, krope = ropes.pop(st)
                sl = slice(st * 128, (st + 1) * 128)
                ptq = psa.tile([128, QH, 128], F32, tag="ptq", bufs=1)
                for h in range(QH):
                    nc.tensor.transpose(ptq[:, h, :], qrope[:, h, :],
                                        identity_f)
                ptk = psa.tile([128, 128], F32, tag="ptk", bufs=1)
                nc.tensor.transpose(ptk, krope, identity_f)
                nc.vector.tensor_copy(qT_all[:, :, sl], ptq)
                nc.vector.tensor_copy(kT_all[:, sl], ptk)

            for st in range(NT):
                xt_t = pa.tile([128, HC, 128], F32R, tag="xt")
                nc.sync.dma_start(out=xt_t, in_=xt4[:, :, st, :])
                cwq_t = pa.tile([128, D], F32, tag="cwq")
                swq_t = pa.tile([128, D], F32, tag="swq")
                cwk_t = pa.tile([128, D], F32, tag="cwk")
                swk_t = pa.tile([128, D], F32, tag="swk")
                sl = slice(st * 128, (st + 1) * 128)
                nc.sync.dma_start(out=cwq_t, in_=cwq[sl, :])
                nc.sync.dma_start(out=swq_t, in_=swq[sl, :])
                nc.sync.dma_start(out=cwk_t, in_=cwk[sl, :])
                nc.sync.dma_start(out=swk_t, in_=swk[sl, :])

                pqkv = psa.tile([128, NW], F32, tag="pqkv", bufs=3)
                for h in range(HC):
                    nc.tensor.matmul(pqkv[:, :DQ], xt_t[:, h, :],
                                     wqkv_sb[:, h, :DQ],
                                     start=(h == 0), stop=(h == HC - 1))
                    nc.tensor.matmul(pqkv[:, DQ:], xt_t[:, h, :],
                                     wqkv_sb[:, h, DQ:],
                                     start=(h == 0), stop=(h == HC - 1))
                q_raw = pa.tile([128, DQ], F32, tag="qraw")
                nc.scalar.copy(q_raw, pqkv[:, :DQ])
                k_raw = pa.tile([128, D], F32, tag="kraw")
                nc.scalar.copy(k_raw, pqkv[:, DQ:DQ + D])
                nc.scalar.copy(v_all[:, st, :], pqkv[:, DQ + D:])

                sq = pa.tile([128, DQ], F32, tag="sq")
                nc.vector.tensor_mul(sq, q_raw, q_raw)
                ssq = pa.tile([128, QH], F32, tag="ssq")
                nc.vector.tensor_reduce(
                    ssq, sq.rearrange("p (h d) -> p h d", d=D),
                    axis=AX.X, op=ALU.add)
                rtq = pa.tile([128, QH], F32, tag="rtq")
                nc.scalar.activation(rtq, ssq, AF.Sqrt, bias=eps_t,
                                     scale=1.0 / D)
                rq = pa.tile([128, QH], F32, tag="rq")
                nc.vector.reciprocal(rq, rtq)

                sqk = pa.tile([128, D], F32, tag="sqk")
                nc.vector.tensor_mul(sqk, k_raw, k_raw)
                ssk = pa.tile([128, 1], F32, tag="ssk")
                nc.vector.tensor_reduce(ssk, sqk, axis=AX.X, op=ALU.add)
                rtk = pa.tile([128, 1], F32, tag="rtk")
                nc.scalar.activation(rtk, ssk, AF.Sqrt, bias=eps_t,
                                     scale=1.0 / D)
                rk = pa.tile([128, 1], F32, tag="rk")
                nc.vector.reciprocal(rk, rtk)

                # rope swaps (half-rotations) of the raw values
                r_q = pa.tile([128, QH, D], F32, tag="rqrot")
                qv = q_raw.rearrange("p (h s d) -> p h s d", h=QH, s=2)
                rv = r_q.rearrange("p h (s d) -> p h s d", s=2)
                nc.gpsimd.tensor_copy(out=rv[:, :, 0, :], in_=qv[:, :, 1, :])
                nc.gpsimd.tensor_copy(out=rv[:, :, 1, :], in_=qv[:, :, 0, :])
                r_k = pa.tile([128, D], F32, tag="rkrot")
                nc.gpsimd.tensor_copy(out=r_k[:, :64], in_=k_raw[:, 64:])
                nc.gpsimd.tensor_copy(out=r_k[:, 64:], in_=k_raw[:, :64])

                qrope = pa.tile([128, QH, D], F32, tag="qrope", bufs=3)
                qh = q_raw.rearrange("p (h d) -> p h d", d=D)
                for h in range(QH):
                    nc.vector.scalar_tensor_tensor(
                        qrope[:, h, :], qh[:, h, :], rq[:, h:h + 1], cwq_t,
                        op0=ALU.mult, op1=ALU.mult)
                    nc.vector.scalar_tensor_tensor(
                        r_q[:, h, :], r_q[:, h, :], rq[:, h:h + 1], swq_t,
                        op0=ALU.mult, op1=ALU.mult)
                nc.gpsimd.tensor_tensor(qrope, qrope, r_q, op=ALU.add)

                krope = pa.tile([128, D], F32, tag="krope", bufs=3)
                nc.vector.scalar_tensor_tensor(krope, k_raw, rk, cwk_t,
                                               op0=ALU.mult, op1=ALU.mult)
                nc.vector.scalar_tensor_tensor(r_k, r_k, rk, swk_t,
                                               op0=ALU.mult, op1=ALU.mult)
                nc.gpsimd.tensor_tensor(krope, krope, r_k, op=ALU.add)

                ropes[st] = (qrope, krope)
                if st >= 2:
                    emit_transpose(st - 2)
            emit_transpose(NT - 2)
            emit_transpose(NT - 1)

          # -------------- phase P-b: gate projection (transposed) ---------
          with tc.tile_pool(name="pb", bufs=1) as pb, \
                tc.tile_pool(name="psb", bufs=2, space="PSUM") as psb:
            for sh in range(2):
                xtb = pb.tile([128, HC, S // 2], F32R, tag="xtb")
                for h in range(HC):
                    nc.sync.dma_start(
                        out=xtb[:, h, :],
                        in_=xt[h * 128:(h + 1) * 128,
                               sh * (S // 2):(sh + 1) * (S // 2)])
                for m in range(QH):
                    pg = psb.tile([128, S // 2], F32, tag="pg")
                    for h in range(HC):
                        for n in range(2):
                            nc.tensor.matmul(
                                pg[:, n * 512:(n + 1) * 512],
                                wg_sb[:, h, m * 128:(m + 1) * 128],
                                xtb[:, h, n * 512:(n + 1) * 512],
                                start=(h == 0), stop=(h == HC - 1))
                    nc.scalar.activation(
                        sigT_all[:, m, sh * (S // 2):(sh + 1) * (S // 2)],
                        pg, AF.Sigmoid)

        # ---------------- attention + gating ------------------------------
        with tc.tile_pool(name="atw", bufs=1) as atw:
            OTg_all = atw.tile([128, QH, S], F32R)
            wo_sb = atw.tile([128, QH, H], F32R)
            if nmix:
                maskt_sb = atw.tile([128, nmix, 256], F32R)
                mt = maskt.rearrange("(m p) q -> p m q", p=128)
                for mi in range(nmix):
                    nc.sync.dma_start(out=maskt_sb[:, mi, :], in_=mt[:, mi, :])

            with tc.tile_pool(name="at", bufs=2) as at, \
                    tc.tile_pool(name="po", bufs=2) as po, \
                    tc.tile_pool(name="ps_st", bufs=2, space="PSUM") as ps_st, \
                    tc.tile_pool(name="ps_ot", bufs=1, space="PSUM") as ps_ot, \
                    tc.tile_pool(name="ps_rb", bufs=1, space="PSUM") as ps_rb, \
                    tc.tile_pool(name="ps_y", bufs=1, space="PSUM") as ps_y:
              for dc in range(QH):
                  nc.sync.dma_start(out=wo_sb[:, dc, :],
                                    in_=wo[dc * 128:(dc + 1) * 128, :])
              for pr in range(NT // 2):
                for h in range(QH):
                    row = rows[pr]
                    nk = len(row)
                    qsl = slice(pr * 256, (pr + 1) * 256)
                    rs_ps = ps_rb.tile([1, 256], F32, tag="rb")
                    ot_ps = ps_ot.tile([128, 256], F32, tag="ot")
                    first = True
                    groups = [row[i:i + MAXGRP] for i in range(0, nk, MAXGRP)]
                    for grp in groups:
                        ng = len(grp)
                        st_ps = ps_st.tile([128, MAXGRP, 256], F32, tag="st")
                        for j, (kj, mi) in enumerate(grp):
                            nc.tensor.matmul(
                                st_ps[:, j, :],
                                kT_all[:, kj * 128:(kj + 1) * 128],
                                qT_all[:, h, qsl],
                                start=True, stop=(mi is None))
                            if mi is not None:
                                nc.tensor.matmul(st_ps[:, j, :], identity_r,
                                                 maskt_sb[:, mi, :],
                                                 start=False, stop=True)
                        est = at.tile([128, MAXGRP, 256], F32R, tag="est", bufs=3)
                        nc.scalar.activation(
                            est[:, :ng, :].rearrange("p g q -> p (g q)"),
                            st_ps[:, :ng, :].rearrange("p g q -> p (g q)"),
                            AF.Exp)
                        for j, (kj, mi) in enumerate(grp):
                            nc.tensor.matmul(rs_ps, ones_col, est[:, j, :],
                                             start=first, stop=False)
                            nc.tensor.matmul(ot_ps, v_all[:, kj, :],
                                             est[:, j, :],
                                             start=first, stop=False)
                            first = False
                    recip = at.tile([1, 256], F32R, tag="recip")
                    nc.vector.reciprocal(recip, rs_ps)
                    bc_ps = ps_rb.tile([128, 256], F32, tag="rb")
                    nc.tensor.matmul(bc_ps, ones_row, recip,
                                     start=True, stop=True)
                    sgr = at.tile([128, 256], F32, tag="sgr")
                    nc.vector.tensor_mul(sgr, bc_ps, sigT_all[:, h, qsl])
                    nc.vector.tensor_mul(OTg_all[:, h, qsl], ot_ps, sgr)

                # output projection for this pair's two s-tiles (keeps PE fed
                # while the next pair's softmax chain runs)
                for st in (2 * pr, 2 * pr + 1):
                    sl = slice(st * 128, (st + 1) * 128)
                    y_sb = po.tile([128, H], F32, tag="ysb")
                    for nh in range(2):
                        py = ps_y.tile([128, H // 2], F32, tag="py")
                        for dc in range(QH):
                            for n2 in range(2):
                                nc.tensor.matmul(
                                    py[:, n2 * 512:(n2 + 1) * 512],
                                    OTg_all[:, dc, sl],
                                    wo_sb[:, dc,
                                          nh * 1024 + n2 * 512:
                                          nh * 1024 + (n2 + 1) * 512],
                                    start=(dc == 0), stop=(dc == QH - 1))
                        if nh == 0:
                            nc.scalar.copy(y_sb[:, :H // 2], py)
                        else:
                            nc.vector.tensor_copy(y_sb[:, H // 2:], py)
                        nc.sync.dma_start(
                            out=y[sl, nh * 1024:(nh + 1) * 1024],
                            in_=y_sb[:, nh * 1024:(nh + 1) * 1024])

    _split_excess_waits(nc)
    return nc


_CACHE = {}
LAST_EXEC_TIME_NS = None
LAST_RESULTS = None


def _maybe_install_profile_hook():
    if not os.environ.get("BASS_TRACE"):
        return
    try:
        import sys
        import types
        import antenv
        if "antenv.axon_hooks" in sys.modules:
            return
        mod = types.ModuleType("antenv.axon_hooks")
        mod._hook = None
        mod.set_axon_ntff_profile_hook = lambda h: setattr(mod, "_hook", h)
        mod.get_axon_ntff_profile_hook = lambda: mod._hook
        sys.modules["antenv.axon_hooks"] = mod
        antenv.axon_hooks = mod
        from trn_agent_boot.trn_boot import _ntff_profile_via_ctypes
        mod.set_axon_ntff_profile_hook(
            _ntff_profile_via_ctypes("/opt/axon/libaxon_pjrt.so"))
    except Exception:
        pass


def kernel(hidden_states, cos, sin, attention_mask, Wq, Wk, Wv, Wo, Wg,
           q_norm_w, k_norm_w):
    global LAST_EXEC_TIME_NS, LAST_RESULTS
    _maybe_install_profile_hook()

    hidden_states = np.asarray(hidden_states, dtype=np.float32)
    cos = np.asarray(cos, dtype=np.float32)
    sin = np.asarray(sin, dtype=np.float32)
    mask2d = np.asarray(attention_mask, dtype=np.float32).reshape(S, S)
    Wq = np.asarray(Wq, dtype=np.float32)
    Wk = np.asarray(Wk, dtype=np.float32)
    Wv = np.asarray(Wv, dtype=np.float32)
    Wo = np.asarray(Wo, dtype=np.float32)
    Wg = np.asarray(Wg, dtype=np.float32)
    qw = np.asarray(q_norm_w, dtype=np.float32)
    kw = np.asarray(k_norm_w, dtype=np.float32)

    rows, mixed = _mask_plan(mask2d)
    nmix = len(mixed)
    plan_key = (tuple(tuple(r) for r in rows), nmix)
    if plan_key not in _CACHE:
        _CACHE[plan_key] = _build(rows, nmix)
    nc = _CACHE[plan_key]

    sign = np.concatenate([-np.ones(D // 2), np.ones(D // 2)]).astype(np.float32)
    qw_swap = np.concatenate([qw[D // 2:], qw[:D // 2]])
    kw_swap = np.concatenate([kw[D // 2:], kw[:D // 2]])
    maskt_np = (np.concatenate(mixed, axis=0) if nmix
                else None)  # [nmix*128, 128]

    in_maps = []
    for c in range(8):
        b, g = divmod(c, 4)
        qs = slice(g * DQ, (g + 1) * DQ)
        ks = slice(g * D, (g + 1) * D)
        m = {
            "xt": np.ascontiguousarray(hidden_states[b].T),
            "wqkv": np.ascontiguousarray(
                np.concatenate([Wq[qs], Wk[ks], Wv[ks]], axis=0).T),
            "wg": np.ascontiguousarray(Wg[qs].T),
            "wo": np.ascontiguousarray(Wo[:, qs].T),
            "cwq": np.ascontiguousarray(cos[b] * qw * LAM),
            "swq": np.ascontiguousarray(sin[b] * (sign * qw_swap) * LAM),
            "cwk": np.ascontiguousarray(cos[b] * kw),
            "swk": np.ascontiguousarray(sin[b] * (sign * kw_swap)),
        }
        if nmix:
            m["maskt"] = maskt_np
        in_maps.append(m)

    res = run_bass_kernel_spmd(nc, in_maps, list(range(8)),
                               trace=bool(os.environ.get("BASS_TRACE")))
    LAST_EXEC_TIME_NS = res.exec_time_ns
    LAST_RESULTS = res

    out = np.empty((B, S, H), dtype=np.float32)
    for b in range(B):
        acc = res.results[4 * b]["y"].astype(np.float32)
        for g in range(1, 4):
            acc = acc + res.results[4 * b + g]["y"]
        out[b] = acc
    return out


# revision 17
# speedup vs baseline: 1.0178x; 1.0178x over previous
"""AFMoE attention layer on 8 NeuronCores (Trainium2, Bass/Tile).

Sharding: core c = (batch b = c//4) x (kv-head group g = c%4).
Each core computes its batch's q-heads 4g..4g+3 + kv head g end-to-end and a
partial output y_c = O_gated @ Wo[:, 512g:512(g+1)].T; the host sums the 4
group partials per batch (row-parallel Wo reduction done on host).

All matmuls run in float32r (full PE rate, ~1e-4 rounding); everything else
is fp32.
"""
import os

import numpy as np

import concourse.bass as bass
import concourse.mybir as mybir
import concourse.tile as tile
from concourse.bass_utils import run_bass_kernel_spmd
from concourse.masks import make_identity

F32 = mybir.dt.float32
F32R = mybir.dt.float32r
AF = mybir.ActivationFunctionType
ALU = mybir.AluOpType
AX = mybir.AxisListType

B, S, H = 2, 2048, 2048
NH, NKV, D = 16, 4, 128
GROUPS = NH // NKV          # q heads per kv head = 4
QH = GROUPS                 # per-core q heads
DQ = QH * D                 # 512
EPS = 1e-5
NT = S // 128               # 16 s-tiles
HC = H // 128               # 16 h-chunks
LAM = float(D) ** -0.5
MAXGRP = 4                  # k-blocks per PSUM score group (x256q = 2 banks)

_nsplit = [0]


def _split_excess_waits(nc, limit=1):
    """This walrus build accepts only one semaphore wait per instruction
    (fp32/fp32r matmuls included). Move excess waits onto preceding
    same-engine NoOps; engine program order keeps this correct."""
    import bass_rust
    for blk in nc.m.functions[0].blocks:
        lst = blk.instructions
        idx = 0
        while idx < len(lst):
            inst = lst[idx]
            si = inst.sync_info
            if (si is None or len(si.on_wait) <= limit
                    or type(inst).__name__ == "InstCollectiveCompute"
                    or inst.engine == mybir.EngineType.Unassigned):
                idx += 1
                continue
            waits = list(si.on_wait)
            kept, excess = waits[-limit:], waits[:-limit]
            new_insts = []
            for w in excess:
                _nsplit[0] += 1
                nop = mybir.InstNoOp(name=f"WS-{_nsplit[0]}", ins=[], outs=[])
                nop.engine = inst.engine
                nop.sync_info = bass_rust.SyncInfo(on_wait=[w], on_update=[])
                new_insts.append(nop)
            inst.sync_info = bass_rust.SyncInfo(on_wait=kept,
                                                on_update=list(si.on_update))
            lst[idx:idx] = new_insts
            idx += len(new_insts) + 1


def _mask_plan(mask2d):
    """Classify the additive mask in [256(q) x 128(k)] slabs (q-tile pairs).

    Returns (rows, mixed_slabs): rows[pair] = list of (kj, mixed_idx|None)
    over a contiguous kj range; mixed_slabs = transposed [128,256] np arrays.
    """
    nb = S // 128
    npair = nb // 2
    uniq = {}
    mixed = []
    rows = []
    for p in range(npair):
        qsl = slice(p * 256, (p + 1) * 256)
        entries = []
        for kj in range(nb):
            blk = mask2d[qsl, kj * 128:(kj + 1) * 128]      # [256 q, 128 k]
            if (blk <= -1e8).all():
                entries.append(None)
            elif (blk == 0.0).all():
                entries.append((kj, None))
            else:
                key = blk.tobytes()
                if key not in uniq:
                    uniq[key] = len(mixed)
                    mixed.append(np.ascontiguousarray(blk.T))
                entries.append((kj, uniq[key]))
        live = [e for e in entries if e is not None]
        if not live:
            raise ValueError("fully-masked query row block unsupported")
        lo = min(e[0] for e in live)
        hi = max(e[0] for e in live)
        row = []
        for kj in range(lo, hi + 1):
            e = entries[kj]
            if e is None:
                blk = mask2d[qsl, kj * 128:(kj + 1) * 128]
                key = blk.tobytes()
                if key not in uniq:
                    uniq[key] = len(mixed)
                    mixed.append(np.ascontiguousarray(blk.T))
                row.append((kj, uniq[key]))
            else:
                row.append(e)
        rows.append(row)
    return rows, mixed


def _build(rows, nmix):
    nc = bass.Bass()
    xt = nc.declare_dram_parameter("xt", [H, S], F32R, isOutput=False)
    wqkv = nc.declare_dram_parameter("wqkv", [H, DQ + 2 * D], F32R, isOutput=False)
    wg = nc.declare_dram_parameter("wg", [H, DQ], F32R, isOutput=False)
    wo = nc.declare_dram_parameter("wo", [DQ, H], F32R, isOutput=False)
    cwq = nc.declare_dram_parameter("cwq", [S, D], F32, isOutput=False)
    swq = nc.declare_dram_parameter("swq", [S, D], F32, isOutput=False)
    cwk = nc.declare_dram_parameter("cwk", [S, D], F32, isOutput=False)
    swk = nc.declare_dram_parameter("swk", [S, D], F32, isOutput=False)
    if nmix:
        maskt = nc.declare_dram_parameter("maskt", [nmix * 128, 256], F32R,
                                          isOutput=False)
    y = nc.declare_dram_parameter("y", [S, H], F32, isOutput=True)

    NW = DQ + 2 * D  # 768

    with tile.TileContext(nc) as tc, \
            nc.allow_low_precision(reason="fp32r matmul operands"), \
            tc.tile_pool(name="const", bufs=1) as const, \
            tc.tile_pool(name="persist", bufs=1) as pp:
        identity_f = const.tile([128, 128], F32)
        make_identity(nc, identity_f)
        if nmix:
            identity_r = const.tile([128, 128], F32R)
            nc.vector.tensor_copy(identity_r, identity_f)
        ones_col_f = const.tile([128, 1], F32)
        nc.vector.memset(ones_col_f, 1.0)
        ones_col = const.tile([128, 1], F32R)
        nc.vector.tensor_copy(ones_col, ones_col_f)
        ones_row_f = const.tile([1, 128], F32)
        nc.vector.memset(ones_row_f, 1.0)
        ones_row = const.tile([1, 128], F32R)
        nc.vector.tensor_copy(ones_row, ones_row_f)
        eps_t = const.tile([128, 1], F32)
        nc.vector.memset(eps_t, EPS)

        qT_all = pp.tile([128, QH, S], F32R)     # [d, h, s]
        kT_all = pp.tile([128, S], F32R)         # [d, s]
        v_all = pp.tile([128, NT, D], F32R)      # [s-part, s-tile, d]
        sigT_all = pp.tile([128, QH, S], F32)    # [d, m, s]

        # ---------------- phase P-a: q/k/v projections + norm + rope ------
        with tc.tile_pool(name="pw", bufs=1) as pw:
          wg_sb = pw.tile([128, HC, DQ], F32R)
          for h in range(HC):
              nc.sync.dma_start(out=wg_sb[:, h, :],
                                in_=wg[h * 128:(h + 1) * 128, :])
          with tc.tile_pool(name="pwq", bufs=1) as pwq, \
                tc.tile_pool(name="pa", bufs=2) as pa, \
                tc.tile_pool(name="psa", bufs=2, space="PSUM") as psa:
            wqkv_sb = pwq.tile([128, HC, NW], F32R)
            for h in range(HC):
                nc.sync.dma_start(out=wqkv_sb[:, h, :],
                                  in_=wqkv[h * 128:(h + 1) * 128, :])
            xt4 = xt.rearrange("(c p) (t q) -> p c t q", p=128, q=128)
            ropes = {}

            def emit_transpose(st):
                qrope, krope = ropes.pop(st)
                sl = slice(st * 128, (st + 1) * 128)
                ptq = psa.tile([128, QH, 128], F32, tag="ptq", bufs=1)
                for h in range(QH):
                    nc.tensor.transpose(ptq[:, h, :], qrope[:, h, :],
                                        identity_f)
                ptk = psa.tile([128, 128], F32, tag="ptk", bufs=1)
                nc.tensor.transpose(ptk, krope, identity_f)
                nc.scalar.copy(qT_all[:, :, sl], ptq)
                nc.scalar.copy(kT_all[:, sl], ptk)

            for st in range(NT):
                xt_t = pa.tile([128, HC, 128], F32R, tag="xt")
                nc.sync.dma_start(out=xt_t, in_=xt4[:, :, st, :])
                cwq_t = pa.tile([128, D], F32, tag="cwq")
                swq_t = pa.tile([128, D], F32, tag="swq")
                cwk_t = pa.tile([128, D], F32, tag="cwk")
                swk_t = pa.tile([128, D], F32, tag="swk")
                sl = slice(st * 128, (st + 1) * 128)
                nc.sync.dma_start(out=cwq_t, in_=cwq[sl, :])
                nc.sync.dma_start(out=swq_t, in_=swq[sl, :])
                nc.sync.dma_start(out=cwk_t, in_=cwk[sl, :])
                nc.sync.dma_start(out=swk_t, in_=swk[sl, :])

                pqkv = psa.tile([128, NW], F32, tag="pqkv", bufs=3)
                for h in range(HC):
                    nc.tensor.matmul(pqkv[:, :DQ], xt_t[:, h, :],
                                     wqkv_sb[:, h, :DQ],
                                     start=(h == 0), stop=(h == HC - 1))
                    nc.tensor.matmul(pqkv[:, DQ:], xt_t[:, h, :],
                                     wqkv_sb[:, h, DQ:],
                                     start=(h == 0), stop=(h == HC - 1))
                q_raw = pa.tile([128, DQ], F32, tag="qraw")
                nc.scalar.copy(q_raw, pqkv[:, :DQ])
                k_raw = pa.tile([128, D], F32, tag="kraw")
                nc.scalar.copy(k_raw, pqkv[:, DQ:DQ + D])
                nc.scalar.copy(v_all[:, st, :], pqkv[:, DQ + D:])

                sq = pa.tile([128, DQ], F32, tag="sq")
                nc.vector.tensor_mul(sq, q_raw, q_raw)
                ssq = pa.tile([128, QH], F32, tag="ssq")
                nc.vector.tensor_reduce(
                    ssq, sq.rearrange("p (h d) -> p h d", d=D),
                    axis=AX.X, op=ALU.add)
                rtq = pa.tile([128, QH], F32, tag="rtq")
                nc.scalar.activation(rtq, ssq, AF.Sqrt, bias=eps_t,
                                     scale=1.0 / D)
                rq = pa.tile([128, QH], F32, tag="rq")
                nc.vector.reciprocal(rq, rtq)

                sqk = pa.tile([128, D], F32, tag="sqk")
                nc.vector.tensor_mul(sqk, k_raw, k_raw)
                ssk = pa.tile([128, 1], F32, tag="ssk")
                nc.vector.tensor_reduce(ssk, sqk, axis=AX.X, op=ALU.add)
                rtk = pa.tile([128, 1], F32, tag="rtk")
                nc.scalar.activation(rtk, ssk, AF.Sqrt, bias=eps_t,
                                     scale=1.0 / D)
                rk = pa.tile([128, 1], F32, tag="rk")
                nc.vector.reciprocal(rk, rtk)

                # rope swaps (half-rotations) of the raw values
                r_q = pa.tile([128, QH, D], F32, tag="rqrot")
                qv = q_raw.rearrange("p (h s d) -> p h s d", h=QH, s=2)
                rv = r_q.rearrange("p h (s d) -> p h s d", s=2)
                nc.gpsimd.tensor_copy(out=rv[:, :, 0, :], in_=qv[:, :, 1, :])
                nc.gpsimd.tensor_copy(out=rv[:, :, 1, :], in_=qv[:, :, 0, :])
                r_k = pa.tile([128, D], F32, tag="rkrot")
                nc.gpsimd.tensor_copy(out=r_k[:, :64], in_=k_raw[:, 64:])
                nc.gpsimd.tensor_copy(out=r_k[:, 64:], in_=k_raw[:, :64])

                qrope = pa.tile([128, QH, D], F32, tag="qrope", bufs=3)
                qh = q_raw.rearrange("p (h d) -> p h d", d=D)
                for h in range(QH):
                    nc.vector.scalar_tensor_tensor(
                        qrope[:, h, :], qh[:, h, :], rq[:, h:h + 1], cwq_t,
                        op0=ALU.mult, op1=ALU.mult)
                    nc.vector.scalar_tensor_tensor(
                        r_q[:, h, :], r_q[:, h, :], rq[:, h:h + 1], swq_t,
                        op0=ALU.mult, op1=ALU.mult)
                nc.gpsimd.tensor_tensor(qrope, qrope, r_q, op=ALU.add)

                krope = pa.tile([128, D], F32, tag="krope", bufs=3)
                nc.vector.scalar_tensor_tensor(krope, k_raw, rk, cwk_t,
                                               op0=ALU.mult, op1=ALU.mult)
                nc.vector.scalar_tensor_tensor(r_k, r_k, rk, swk_t,
                                               op0=ALU.mult, op1=ALU.mult)
                nc.gpsimd.tensor_tensor(krope, krope, r_k, op=ALU.add)

                ropes[st] = (qrope, krope)
                if st >= 2:
                    emit_transpose(st - 2)
            emit_transpose(NT - 2)
            emit_transpose(NT - 1)

          # -------------- phase P-b: gate projection (transposed) ---------
          with tc.tile_pool(name="pb", bufs=1) as pb, \
                tc.tile_pool(name="psb", bufs=2, space="PSUM") as psb:
            for sh in range(2):
                xtb = pb.tile([128, HC, S // 2], F32R, tag="xtb")
                for h in range(HC):
                    nc.sync.dma_start(
                        out=xtb[:, h, :],
                        in_=xt[h * 128:(h + 1) * 128,
                               sh * (S // 2):(sh + 1) * (S // 2)])
                for m in range(QH):
                    pg = psb.tile([128, S // 2], F32, tag="pg")
                    for h in range(HC):
                        for n in range(2):
                            nc.tensor.matmul(
                                pg[:, n * 512:(n + 1) * 512],
                                wg_sb[:, h, m * 128:(m + 1) * 128],
                                xtb[:, h, n * 512:(n + 1) * 512],
                                start=(h == 0), stop=(h == HC - 1))
                    nc.scalar.activation(
                        sigT_all[:, m, sh * (S // 2):(sh + 1) * (S // 2)],
                        pg, AF.Sigmoid)

        # ---------------- attention + gating ------------------------------
        with tc.tile_pool(name="atw", bufs=1) as atw:
            OTg_all = atw.tile([128, QH, S], F32R)
            wo_sb = atw.tile([128, QH, H], F32R)
            if nmix:
                maskt_sb = atw.tile([128, nmix, 256], F32R)
                mt = maskt.rearrange("(m p) q -> p m q", p=128)
                for mi in range(nmix):
                    nc.sync.dma_start(out=maskt_sb[:, mi, :], in_=mt[:, mi, :])

            with tc.tile_pool(name="at", bufs=2) as at, \
                    tc.tile_pool(name="po", bufs=2) as po, \
                    tc.tile_pool(name="ps_st", bufs=2, space="PSUM") as ps_st, \
                    tc.tile_pool(name="ps_ot", bufs=1, space="PSUM") as ps_ot, \
                    tc.tile_pool(name="ps_rb", bufs=1, space="PSUM") as ps_rb, \
                    tc.tile_pool(name="ps_y", bufs=1, space="PSUM") as ps_y:
              def emit_wo(pr):
                for st in (2 * pr, 2 * pr + 1):
                    sl = slice(st * 128, (st + 1) * 128)
                    y_sb = po.tile([128, H], F32, tag="ysb")
                    for nh in range(2):
                        py = ps_y.tile([128, H // 2], F32, tag="py")
                        for dc in range(QH):
                            for n2 in range(2):
                                nc.tensor.matmul(
                                    py[:, n2 * 512:(n2 + 1) * 512],
                                    OTg_all[:, dc, sl],
                                    wo_sb[:, dc,
                                          nh * 1024 + n2 * 512:
                                          nh * 1024 + (n2 + 1) * 512],
                                    start=(dc == 0), stop=(dc == QH - 1))
                        if nh == 0:
                            nc.scalar.copy(y_sb[:, :H // 2], py)
                        else:
                            nc.vector.tensor_copy(y_sb[:, H // 2:], py)
                        nc.sync.dma_start(
                            out=y[sl, nh * 1024:(nh + 1) * 1024],
                            in_=y_sb[:, nh * 1024:(nh + 1) * 1024])

              WOLAG = 3
              for pr in range(NT // 2):
                if pr < QH:
                    nc.sync.dma_start(out=wo_sb[:, pr, :],
                                      in_=wo[pr * 128:(pr + 1) * 128, :])
                for h in range(QH):
                    row = rows[pr]
                    nk = len(row)
                    qsl = slice(pr * 256, (pr + 1) * 256)
                    rs_ps = ps_rb.tile([1, 256], F32, tag="rb")
                    ot_ps = ps_ot.tile([128, 256], F32, tag="ot")
                    first = True
                    groups = [row[i:i + MAXGRP] for i in range(0, nk, MAXGRP)]
                    for grp in groups:
                        ng = len(grp)
                        st_ps = ps_st.tile([128, MAXGRP, 256], F32, tag="st")
                        for j, (kj, mi) in enumerate(grp):
                            nc.tensor.matmul(
                                st_ps[:, j, :],
                                kT_all[:, kj * 128:(kj + 1) * 128],
                                qT_all[:, h, qsl],
                                start=True, stop=(mi is None))
                            if mi is not None:
                                nc.tensor.matmul(st_ps[:, j, :], identity_r,
                                                 maskt_sb[:, mi, :],
                                                 start=False, stop=True)
                        est = at.tile([128, MAXGRP, 256], F32R, tag="est", bufs=3)
                        nc.scalar.activation(
                            est[:, :ng, :].rearrange("p g q -> p (g q)"),
                            st_ps[:, :ng, :].rearrange("p g q -> p (g q)"),
                            AF.Exp)
                        for j, (kj, mi) in enumerate(grp):
                            nc.tensor.matmul(rs_ps, ones_col, est[:, j, :],
                                             start=first, stop=False)
                            nc.tensor.matmul(ot_ps, v_all[:, kj, :],
                                             est[:, j, :],
                                             start=first, stop=False)
                            first = False
                    recip = at.tile([1, 256], F32R, tag="recip")
                    nc.vector.reciprocal(recip, rs_ps)
                    bc_ps = ps_rb.tile([128, 256], F32, tag="rb")
                    nc.tensor.matmul(bc_ps, ones_row, recip,
                                     start=True, stop=True)
                    sgr = at.tile([128, 256], F32, tag="sgr")
                    nc.vector.tensor_mul(sgr, bc_ps, sigT_all[:, h, qsl])
                    nc.vector.tensor_mul(OTg_all[:, h, qsl], ot_ps, sgr)

                if pr >= WOLAG:
                    emit_wo(pr - WOLAG)
              for pr in range(NT // 2 - WOLAG, NT // 2):
                emit_wo(pr)

    _split_excess_waits(nc)
    return nc


_CACHE = {}
LAST_EXEC_TIME_NS = None
LAST_RESULTS = None


def _maybe_install_profile_hook():
    if not os.environ.get("BASS_TRACE"):
        return
    try:
        import sys
        import types
        import antenv
        if "antenv.axon_hooks" in sys.modules:
            return
        mod = types.ModuleType("antenv.axon_hooks")
        mod._hook = None
        mod.set_axon_ntff_profile_hook = lambda h: setattr(mod, "_hook", h)
        mod.get_axon_ntff_profile_hook = lambda: mod._hook
        sys.modules["antenv.axon_hooks"] = mod
        antenv.axon_hooks = mod
        from trn_agent_boot.trn_boot import _ntff_profile_via_ctypes
        mod.set_axon_ntff_profile_hook(
            _ntff_profile_via_ctypes("/opt/axon/libaxon_pjrt.so"))
    except Exception:
        pass


def kernel(hidden_states, cos, sin, attention_mask, Wq, Wk, Wv, Wo, Wg,
           q_norm_w, k_norm_w):
    global LAST_EXEC_TIME_NS, LAST_RESULTS
    _maybe_install_profile_hook()

    hidden_states = np.asarray(hidden_states, dtype=np.float32)
    cos = np.asarray(cos, dtype=np.float32)
    sin = np.asarray(sin, dtype=np.float32)
    mask2d = np.asarray(attention_mask, dtype=np.float32).reshape(S, S)
    Wq = np.asarray(Wq, dtype=np.float32)
    Wk = np.asarray(Wk, dtype=np.float32)
    Wv = np.asarray(Wv, dtype=np.float32)
    Wo = np.asarray(Wo, dtype=np.float32)
    Wg = np.asarray(Wg, dtype=np.float32)
    qw = np.asarray(q_norm_w, dtype=np.float32)
    kw = np.asarray(k_norm_w, dtype=np.float32)

    rows, mixed = _mask_plan(mask2d)
    nmix = len(mixed)
    plan_key = (tuple(tuple(r) for r in rows), nmix)
    if plan_key not in _CACHE:
        _CACHE[plan_key] = _build(rows, nmix)
    nc = _CACHE[plan_key]

    sign = np.concatenate([-np.ones(D // 2), np.ones(D // 2)]).astype(np.float32)
    qw_swap = np.concatenate([qw[D // 2:], qw[:D // 2]])
    kw_swap = np.concatenate([kw[D // 2:], kw[:D // 2]])
    maskt_np = (np.concatenate(mixed, axis=0) if nmix
                else None)  # [nmix*128, 128]

    in_maps = []
    for c in range(8):
        b, g = divmod(c, 4)
        qs = slice(g * DQ, (g + 1) * DQ)
        ks = slice(g * D, (g + 1) * D)
        m = {
            "xt": np.ascontiguousarray(hidden_states[b].T),
            "wqkv": np.ascontiguousarray(
                np.concatenate([Wq[qs], Wk[ks], Wv[ks]], axis=0).T),
            "wg": np.ascontiguousarray(Wg[qs].T),
            "wo": np.ascontiguousarray(Wo[:, qs].T),
            "cwq": np.ascontiguousarray(cos[b] * qw * LAM),
            "swq": np.ascontiguousarray(sin[b] * (sign * qw_swap) * LAM),
            "cwk": np.ascontiguousarray(cos[b] * kw),
            "swk": np.ascontiguousarray(sin[b] * (sign * kw_swap)),
        }
        if nmix:
            m["maskt"] = maskt_np
        in_maps.append(m)

    res = run_bass_kernel_spmd(nc, in_maps, list(range(8)),
                               trace=bool(os.environ.get("BASS_TRACE")))
    LAST_EXEC_TIME_NS = res.exec_time_ns
    LAST_RESULTS = res

    out = np.empty((B, S, H), dtype=np.float32)
    for b in range(B):
        acc = res.results[4 * b]["y"].astype(np.float32)
        for g in range(1, 4):
            acc = acc + res.results[4 * b + g]["y"]
        out[b] = acc
    return out


# revision 18
# speedup vs baseline: 1.0537x; 1.0353x over previous
"""AFMoE attention layer on 8 NeuronCores (Trainium2, Bass/Tile).

Sharding: core c = (batch b = c//4) x (kv-head group g = c%4).
Each core computes its batch's q-heads 4g..4g+3 + kv head g end-to-end and a
partial output y_c = O_gated @ Wo[:, 512g:512(g+1)].T; the host sums the 4
group partials per batch (row-parallel Wo reduction done on host).

All matmuls run in float32r (full PE rate, ~1e-4 rounding); everything else
is fp32.
"""
import os

import numpy as np

import concourse.bass as bass
import concourse.mybir as mybir
import concourse.tile as tile
from concourse.bass_utils import run_bass_kernel_spmd
from concourse.masks import make_identity

F32 = mybir.dt.float32
F32R = mybir.dt.float32r
AF = mybir.ActivationFunctionType
ALU = mybir.AluOpType
AX = mybir.AxisListType

B, S, H = 2, 2048, 2048
NH, NKV, D = 16, 4, 128
GROUPS = NH // NKV          # q heads per kv head = 4
QH = GROUPS                 # per-core q heads
DQ = QH * D                 # 512
EPS = 1e-5
NT = S // 128               # 16 s-tiles
HC = H // 128               # 16 h-chunks
LAM = float(D) ** -0.5
MAXGRP = 4                  # k-blocks per PSUM score group (x256q = 2 banks)

_nsplit = [0]


def _split_excess_waits(nc, limit=1):
    """This walrus build accepts only one semaphore wait per instruction
    (fp32/fp32r matmuls included). Move excess waits onto preceding
    same-engine NoOps; engine program order keeps this correct."""
    import bass_rust
    for blk in nc.m.functions[0].blocks:
        lst = blk.instructions
        idx = 0
        while idx < len(lst):
            inst = lst[idx]
            si = inst.sync_info
            if (si is None or len(si.on_wait) <= limit
                    or type(inst).__name__ == "InstCollectiveCompute"
                    or inst.engine == mybir.EngineType.Unassigned):
                idx += 1
                continue
            waits = list(si.on_wait)
            kept, excess = waits[-limit:], waits[:-limit]
            new_insts = []
            for w in excess:
                _nsplit[0] += 1
                nop = mybir.InstNoOp(name=f"WS-{_nsplit[0]}", ins=[], outs=[])
                nop.engine = inst.engine
                nop.sync_info = bass_rust.SyncInfo(on_wait=[w], on_update=[])
                new_insts.append(nop)
            inst.sync_info = bass_rust.SyncInfo(on_wait=kept,
                                                on_update=list(si.on_update))
            lst[idx:idx] = new_insts
            idx += len(new_insts) + 1


def _mask_plan(mask2d):
    """Classify the additive mask in [256(q) x 128(k)] slabs (q-tile pairs).

    Returns (rows, mixed_slabs): rows[pair] = list of (kj, mixed_idx|None)
    over a contiguous kj range; mixed_slabs = transposed [128,256] np arrays.
    """
    nb = S // 128
    npair = nb // 2
    uniq = {}
    mixed = []
    rows = []
    for p in range(npair):
        qsl = slice(p * 256, (p + 1) * 256)
        entries = []
        for kj in range(nb):
            blk = mask2d[qsl, kj * 128:(kj + 1) * 128]      # [256 q, 128 k]
            if (blk <= -1e8).all():
                entries.append(None)
            elif (blk == 0.0).all():
                entries.append((kj, None))
            else:
                key = blk.tobytes()
                if key not in uniq:
                    uniq[key] = len(mixed)
                    mixed.append(np.ascontiguousarray(blk.T))
                entries.append((kj, uniq[key]))
        live = [e for e in entries if e is not None]
        if not live:
            raise ValueError("fully-masked query row block unsupported")
        lo = min(e[0] for e in live)
        hi = max(e[0] for e in live)
        row = []
        for kj in range(lo, hi + 1):
            e = entries[kj]
            if e is None:
                blk = mask2d[qsl, kj * 128:(kj + 1) * 128]
                key = blk.tobytes()
                if key not in uniq:
                    uniq[key] = len(mixed)
                    mixed.append(np.ascontiguousarray(blk.T))
                row.append((kj, uniq[key]))
            else:
                row.append(e)
        rows.append(row)
    return rows, mixed


def _build(rows, nmix):
    nc = bass.Bass()
    xt = nc.declare_dram_parameter("xt", [H, S], F32R, isOutput=False)
    wqkv = nc.declare_dram_parameter("wqkv", [H, DQ + 2 * D], F32R, isOutput=False)
    wg = nc.declare_dram_parameter("wg", [H, DQ], F32R, isOutput=False)
    wo = nc.declare_dram_parameter("wo", [DQ, H], F32R, isOutput=False)
    cwq = nc.declare_dram_parameter("cwq", [S, D], F32, isOutput=False)
    swq = nc.declare_dram_parameter("swq", [S, D], F32, isOutput=False)
    cwk = nc.declare_dram_parameter("cwk", [S, D], F32, isOutput=False)
    swk = nc.declare_dram_parameter("swk", [S, D], F32, isOutput=False)
    if nmix:
        maskt = nc.declare_dram_parameter("maskt", [nmix * 128, 256], F32R,
                                          isOutput=False)
    y = nc.declare_dram_parameter("y", [S, H], F32, isOutput=True)

    NW = DQ + 2 * D  # 768

    with tile.TileContext(nc) as tc, \
            nc.allow_low_precision(reason="fp32r matmul operands"), \
            tc.tile_pool(name="const", bufs=1) as const, \
            tc.tile_pool(name="persist", bufs=1) as pp:
        identity_f = const.tile([128, 128], F32)
        make_identity(nc, identity_f)
        if nmix:
            identity_r = const.tile([128, 128], F32R)
            nc.vector.tensor_copy(identity_r, identity_f)
        ones_col_f = const.tile([128, 1], F32)
        nc.vector.memset(ones_col_f, 1.0)
        ones_col = const.tile([128, 1], F32R)
        nc.vector.tensor_copy(ones_col, ones_col_f)
        ones_row_f = const.tile([1, 128], F32)
        nc.vector.memset(ones_row_f, 1.0)
        ones_row = const.tile([1, 128], F32R)
        nc.vector.tensor_copy(ones_row, ones_row_f)
        eps_t = const.tile([128, 1], F32)
        nc.vector.memset(eps_t, EPS)

        qT_all = pp.tile([128, QH, S], F32R)     # [d, h, s]
        kT_all = pp.tile([128, S], F32R)         # [d, s]
        v_all = pp.tile([128, NT, D], F32R)      # [s-part, s-tile, d]
        sigT_all = pp.tile([128, QH, S], F32)    # [d, m, s]

        # ---------------- phase P-a: q/k/v projections + norm + rope ------
        with tc.tile_pool(name="pw", bufs=1) as pw:
          wg_sb = pw.tile([128, HC, DQ], F32R)
          for h in range(HC):
              nc.sync.dma_start(out=wg_sb[:, h, :],
                                in_=wg[h * 128:(h + 1) * 128, :])
          with tc.tile_pool(name="pwq", bufs=1) as pwq, \
                tc.tile_pool(name="pa", bufs=2) as pa, \
                tc.tile_pool(name="psa", bufs=2, space="PSUM") as psa:
            wqkv_sb = pwq.tile([128, HC, NW], F32R)
            for h in range(HC):
                nc.sync.dma_start(out=wqkv_sb[:, h, :],
                                  in_=wqkv[h * 128:(h + 1) * 128, :])
            xt4 = xt.rearrange("(c p) (t q) -> p c t q", p=128, q=128)
            ropes = {}

            def emit_transpose(st):
                qrope, krope = ropes.pop(st)
                sl = slice(st * 128, (st + 1) * 128)
                ptq = psa.tile([128, QH, 128], F32, tag="ptq", bufs=1)
                for h in range(QH):
                    nc.tensor.transpose(ptq[:, h, :], qrope[:, h, :],
                                        identity_f)
                ptk = psa.tile([128, 128], F32, tag="ptk", bufs=1)
                nc.tensor.transpose(ptk, krope, identity_f)
                nc.scalar.copy(qT_all[:, :, sl], ptq)
                nc.scalar.copy(kT_all[:, sl], ptk)

            for st in range(NT):
                xt_t = pa.tile([128, HC, 128], F32R, tag="xt")
                nc.sync.dma_start(out=xt_t, in_=xt4[:, :, st, :])
                cwq_t = pa.tile([128, D], F32, tag="cwq")
                swq_t = pa.tile([128, D], F32, tag="swq")
                cwk_t = pa.tile([128, D], F32, tag="cwk")
                swk_t = pa.tile([128, D], F32, tag="swk")
                sl = slice(st * 128, (st + 1) * 128)
                nc.sync.dma_start(out=cwq_t, in_=cwq[sl, :])
                nc.sync.dma_start(out=swq_t, in_=swq[sl, :])
                nc.sync.dma_start(out=cwk_t, in_=cwk[sl, :])
                nc.sync.dma_start(out=swk_t, in_=swk[sl, :])

                pqkv = psa.tile([128, NW], F32, tag="pqkv", bufs=3)
                for h in range(HC):
                    nc.tensor.matmul(pqkv[:, :DQ], xt_t[:, h, :],
                                     wqkv_sb[:, h, :DQ],
                                     start=(h == 0), stop=(h == HC - 1))
                    nc.tensor.matmul(pqkv[:, DQ:], xt_t[:, h, :],
                                     wqkv_sb[:, h, DQ:],
                                     start=(h == 0), stop=(h == HC - 1))
                q_raw = pa.tile([128, DQ], F32, tag="qraw")
                nc.scalar.copy(q_raw, pqkv[:, :DQ])
                k_raw = pa.tile([128, D], F32, tag="kraw")
                nc.scalar.copy(k_raw, pqkv[:, DQ:DQ + D])
                nc.scalar.copy(v_all[:, st, :], pqkv[:, DQ + D:])

                sq = pa.tile([128, DQ], F32, tag="sq")
                nc.vector.tensor_mul(sq, q_raw, q_raw)
                ssq = pa.tile([128, QH], F32, tag="ssq")
                nc.vector.tensor_reduce(
                    ssq, sq.rearrange("p (h d) -> p h d", d=D),
                    axis=AX.X, op=ALU.add)
                rtq = pa.tile([128, QH], F32, tag="rtq")
                nc.scalar.activation(rtq, ssq, AF.Sqrt, bias=eps_t,
                                     scale=1.0 / D)
                rq = pa.tile([128, QH], F32, tag="rq")
                nc.vector.reciprocal(rq, rtq)

                sqk = pa.tile([128, D], F32, tag="sqk")
                nc.vector.tensor_mul(sqk, k_raw, k_raw)
                ssk = pa.tile([128, 1], F32, tag="ssk")
                nc.vector.tensor_reduce(ssk, sqk, axis=AX.X, op=ALU.add)
                rtk = pa.tile([128, 1], F32, tag="rtk")
                nc.scalar.activation(rtk, ssk, AF.Sqrt, bias=eps_t,
                                     scale=1.0 / D)
                rk = pa.tile([128, 1], F32, tag="rk")
                nc.vector.reciprocal(rk, rtk)

                # rope swaps (half-rotations) of the raw values
                r_q = pa.tile([128, QH, D], F32, tag="rqrot")
                qv = q_raw.rearrange("p (h s d) -> p h s d", h=QH, s=2)
                rv = r_q.rearrange("p h (s d) -> p h s d", s=2)
                nc.gpsimd.tensor_copy(out=rv[:, :, 0, :], in_=qv[:, :, 1, :])
                nc.gpsimd.tensor_copy(out=rv[:, :, 1, :], in_=qv[:, :, 0, :])
                r_k = pa.tile([128, D], F32, tag="rkrot")
                nc.gpsimd.tensor_copy(out=r_k[:, :64], in_=k_raw[:, 64:])
                nc.gpsimd.tensor_copy(out=r_k[:, 64:], in_=k_raw[:, :64])

                qrope = pa.tile([128, QH, D], F32, tag="qrope", bufs=3)
                qh = q_raw.rearrange("p (h d) -> p h d", d=D)
                for h in range(QH):
                    nc.vector.scalar_tensor_tensor(
                        qrope[:, h, :], qh[:, h, :], rq[:, h:h + 1], cwq_t,
                        op0=ALU.mult, op1=ALU.mult)
                    nc.vector.scalar_tensor_tensor(
                        r_q[:, h, :], r_q[:, h, :], rq[:, h:h + 1], swq_t,
                        op0=ALU.mult, op1=ALU.mult)
                nc.gpsimd.tensor_tensor(qrope, qrope, r_q, op=ALU.add)

                krope = pa.tile([128, D], F32, tag="krope", bufs=3)
                nc.vector.scalar_tensor_tensor(krope, k_raw, rk, cwk_t,
                                               op0=ALU.mult, op1=ALU.mult)
                nc.vector.scalar_tensor_tensor(r_k, r_k, rk, swk_t,
                                               op0=ALU.mult, op1=ALU.mult)
                nc.gpsimd.tensor_tensor(krope, krope, r_k, op=ALU.add)

                ropes[st] = (qrope, krope)
                if st >= 2:
                    emit_transpose(st - 2)
            emit_transpose(NT - 2)
            emit_transpose(NT - 1)

          # -------------- phase P-b: gate projection (transposed) ---------
          with tc.tile_pool(name="pb", bufs=1) as pb, \
                tc.tile_pool(name="psb", bufs=2, space="PSUM") as psb:
            for sh in range(2):
                xtb = pb.tile([128, HC, S // 2], F32R, tag="xtb")
                for h in range(HC):
                    nc.sync.dma_start(
                        out=xtb[:, h, :],
                        in_=xt[h * 128:(h + 1) * 128,
                               sh * (S // 2):(sh + 1) * (S // 2)])
                for m in range(QH):
                    pg = psb.tile([128, S // 2], F32, tag="pg")
                    for h in range(HC):
                        for n in range(2):
                            nc.tensor.matmul(
                                pg[:, n * 512:(n + 1) * 512],
                                wg_sb[:, h, m * 128:(m + 1) * 128],
                                xtb[:, h, n * 512:(n + 1) * 512],
                                start=(h == 0), stop=(h == HC - 1))
                    nc.scalar.activation(
                        sigT_all[:, m, sh * (S // 2):(sh + 1) * (S // 2)],
                        pg, AF.Sigmoid)

        # ---------------- attention + gating ------------------------------
        with tc.tile_pool(name="atw", bufs=1) as atw:
            OTg_all = atw.tile([128, QH, S], F32R)
            wo_sb = atw.tile([128, QH, H], F32R)
            if nmix:
                maskt_sb = atw.tile([128, nmix, 256], F32R)
                mt = maskt.rearrange("(m p) q -> p m q", p=128)
                for mi in range(nmix):
                    nc.sync.dma_start(out=maskt_sb[:, mi, :], in_=mt[:, mi, :])

            with tc.tile_pool(name="at", bufs=2) as at, \
                    tc.tile_pool(name="po", bufs=2) as po, \
                    tc.tile_pool(name="ps_st", bufs=2, space="PSUM") as ps_st, \
                    tc.tile_pool(name="ps_ot", bufs=1, space="PSUM") as ps_ot, \
                    tc.tile_pool(name="ps_rb", bufs=1, space="PSUM") as ps_rb, \
                    tc.tile_pool(name="ps_y", bufs=1, space="PSUM") as ps_y:
              def emit_wo(pr):
                for st in (2 * pr, 2 * pr + 1):
                    sl = slice(st * 128, (st + 1) * 128)
                    y_sb = po.tile([128, H], F32, tag="ysb")
                    for nh in range(2):
                        py = ps_y.tile([128, H // 2], F32, tag="py")
                        for dc in range(QH):
                            for n2 in range(2):
                                nc.tensor.matmul(
                                    py[:, n2 * 512:(n2 + 1) * 512],
                                    OTg_all[:, dc, sl],
                                    wo_sb[:, dc,
                                          nh * 1024 + n2 * 512:
                                          nh * 1024 + (n2 + 1) * 512],
                                    start=(dc == 0), stop=(dc == QH - 1))
                        if nh == 0:
                            nc.scalar.copy(y_sb[:, :H // 2], py)
                        else:
                            nc.vector.tensor_copy(y_sb[:, H // 2:], py)
                        nc.sync.dma_start(
                            out=y[sl, nh * 1024:(nh + 1) * 1024],
                            in_=y_sb[:, nh * 1024:(nh + 1) * 1024])

              WOLAG = 1
              for dc in range(QH):
                  nc.sync.dma_start(out=wo_sb[:, dc, :],
                                    in_=wo[dc * 128:(dc + 1) * 128, :])
              for pr in range(NT // 2):
                for h in range(QH):
                    row = rows[pr]
                    nk = len(row)
                    qsl = slice(pr * 256, (pr + 1) * 256)
                    rs_ps = ps_rb.tile([1, 256], F32, tag="rb")
                    ot_ps = ps_ot.tile([128, 256], F32, tag="ot")
                    first = True
                    groups = [row[i:i + MAXGRP] for i in range(0, nk, MAXGRP)]
                    for grp in groups:
                        ng = len(grp)
                        st_ps = ps_st.tile([128, MAXGRP, 256], F32, tag="st")
                        for j, (kj, mi) in enumerate(grp):
                            nc.tensor.matmul(
                                st_ps[:, j, :],
                                kT_all[:, kj * 128:(kj + 1) * 128],
                                qT_all[:, h, qsl],
                                start=True, stop=(mi is None))
                            if mi is not None:
                                nc.tensor.matmul(st_ps[:, j, :], identity_r,
                                                 maskt_sb[:, mi, :],
                                                 start=False, stop=True)
                        est = at.tile([128, MAXGRP, 256], F32R, tag="est", bufs=3)
                        nc.scalar.activation(
                            est[:, :ng, :].rearrange("p g q -> p (g q)"),
                            st_ps[:, :ng, :].rearrange("p g q -> p (g q)"),
                            AF.Exp)
                        for j, (kj, mi) in enumerate(grp):
                            nc.tensor.matmul(rs_ps, ones_col, est[:, j, :],
                                             start=first, stop=False)
                            nc.tensor.matmul(ot_ps, v_all[:, kj, :],
                                             est[:, j, :],
                                             start=first, stop=False)
                            first = False
                    recip = at.tile([1, 256], F32R, tag="recip")
                    nc.vector.reciprocal(recip, rs_ps)
                    bc_ps = ps_rb.tile([128, 256], F32, tag="rb")
                    nc.tensor.matmul(bc_ps, ones_row, recip,
                                     start=True, stop=True)
                    sgr = at.tile([128, 256], F32, tag="sgr")
                    nc.vector.tensor_mul(sgr, bc_ps, sigT_all[:, h, qsl])
                    nc.vector.tensor_mul(OTg_all[:, h, qsl], ot_ps, sgr)

                if pr >= WOLAG:
                    emit_wo(pr - WOLAG)
              for pr in range(NT // 2 - WOLAG, NT // 2):
                emit_wo(pr)

    _split_excess_waits(nc)
    return nc


_CACHE = {}
LAST_EXEC_TIME_NS = None
LAST_RESULTS = None


def _maybe_install_profile_hook():
    if not os.environ.get("BASS_TRACE"):
        return
    try:
        import sys
        import types
        import antenv
        if "antenv.axon_hooks" in sys.modules:
            return
        mod = types.ModuleType("antenv.axon_hooks")
        mod._hook = None
        mod.set_axon_ntff_profile_hook = lambda h: setattr(mod, "_hook", h)
        mod.get_axon_ntff_profile_hook = lambda: mod._hook
        sys.modules["antenv.axon_hooks"] = mod
        antenv.axon_hooks = mod
        from trn_agent_boot.trn_boot import _ntff_profile_via_ctypes
        mod.set_axon_ntff_profile_hook(
            _ntff_profile_via_ctypes("/opt/axon/libaxon_pjrt.so"))
    except Exception:
        pass


def kernel(hidden_states, cos, sin, attention_mask, Wq, Wk, Wv, Wo, Wg,
           q_norm_w, k_norm_w):
    global LAST_EXEC_TIME_NS, LAST_RESULTS
    _maybe_install_profile_hook()

    hidden_states = np.asarray(hidden_states, dtype=np.float32)
    cos = np.asarray(cos, dtype=np.float32)
    sin = np.asarray(sin, dtype=np.float32)
    mask2d = np.asarray(attention_mask, dtype=np.float32).reshape(S, S)
    Wq = np.asarray(Wq, dtype=np.float32)
    Wk = np.asarray(Wk, dtype=np.float32)
    Wv = np.asarray(Wv, dtype=np.float32)
    Wo = np.asarray(Wo, dtype=np.float32)
    Wg = np.asarray(Wg, dtype=np.float32)
    qw = np.asarray(q_norm_w, dtype=np.float32)
    kw = np.asarray(k_norm_w, dtype=np.float32)

    rows, mixed = _mask_plan(mask2d)
    nmix = len(mixed)
    plan_key = (tuple(tuple(r) for r in rows), nmix)
    if plan_key not in _CACHE:
        _CACHE[plan_key] = _build(rows, nmix)
    nc = _CACHE[plan_key]

    sign = np.concatenate([-np.ones(D // 2), np.ones(D // 2)]).astype(np.float32)
    qw_swap = np.concatenate([qw[D // 2:], qw[:D // 2]])
    kw_swap = np.concatenate([kw[D // 2:], kw[:D // 2]])
    maskt_np = (np.concatenate(mixed, axis=0) if nmix
                else None)  # [nmix*128, 128]

    in_maps = []
    for c in range(8):
        b, g = divmod(c, 4)
        qs = slice(g * DQ, (g + 1) * DQ)
        ks = slice(g * D, (g + 1) * D)
        m = {
            "xt": np.ascontiguousarray(hidden_states[b].T),
            "wqkv": np.ascontiguousarray(
                np.concatenate([Wq[qs], Wk[ks], Wv[ks]], axis=0).T),
            "wg": np.ascontiguousarray(Wg[qs].T),
            "wo": np.ascontiguousarray(Wo[:, qs].T),
            "cwq": np.ascontiguousarray(cos[b] * qw * LAM),
            "swq": np.ascontiguousarray(sin[b] * (sign * qw_swap) * LAM),
            "cwk": np.ascontiguousarray(cos[b] * kw),
            "swk": np.ascontiguousarray(sin[b] * (sign * kw_swap)),
        }
        if nmix:
            m["maskt"] = maskt_np
        in_maps.append(m)

    res = run_bass_kernel_spmd(nc, in_maps, list(range(8)),
                               trace=bool(os.environ.get("BASS_TRACE")))
    LAST_EXEC_TIME_NS = res.exec_time_ns
    LAST_RESULTS = res

    out = np.empty((B, S, H), dtype=np.float32)
    for b in range(B):
        acc = res.results[4 * b]["y"].astype(np.float32)
        for g in range(1, 4):
            acc = acc + res.results[4 * b + g]["y"]
        out[b] = acc
    return out
